# revision 35
# baseline (speedup 1.0000x reference)
"""Trainium2 Bass kernel for a pre-LN transformer block with cosFormer linear
attention (B=4, S=8192, D=768, H=12, FF=3072) on 8 NeuronCores.

Sharding: core c handles batch c//2, sequence half c%2 (T=4096 tokens).
Cross-core communication: one small AllReduce of the per-(batch,head) kv/ksum
statistics ([128, 12*65] f32 ~ 400KB) between core pairs sharing a batch.

Layout strategy:
  - token-major [tok, feat] for LayerNorm stats / per-token scalars (z, keep)
  - feature-major [feat, tok] as matmul inputs (lhsT/rhs), produced either
    directly (weights-stationary) or via PE-transpose of 128x128 blocks
  - mask folded into v (exact: kv/ksum terms are zeroed identically)
  - ksum obtained as a 65th "keep" column of v_aug in the same matmul as kv
  - LN affine params folded into the following projection weights on host
  - bf16 matmul inputs, f32 psum accumulation and residuals

Device outputs per core: x2 (x + attn block output, token-major f32) and
y_fm (FFN output, feature-major f32). Host computes out = x2 + y_fm.T.
"""

import os
import numpy as np
import ml_dtypes

import concourse.bass as bass
import concourse.tile as tile
from concourse import bacc, mybir
from concourse import bass_utils
from concourse.bass import ds, ts
from concourse.masks import make_identity

BF16 = mybir.dt.bfloat16
F32 = mybir.dt.float32
AF = mybir.ActivationFunctionType
ALU = mybir.AluOpType

B, S, D, H = 4, 8192, 768, 12
DH = D // H            # 64
FF = 4 * D             # 3072
LN_EPS = 1e-5
DENOM_EPS = 1e-5

NCORES = 8
T = (B * S) // NCORES  # 4096 tokens per core
P = 128
NT = T // P            # 32 token tiles
KD = D // P            # 6 feature chunks of 128
KF = FF // P           # 24 ffn chunks of 128
NQ = T // 512          # 8 moving chunks of 512 tokens

_CACHE = {}
LAST_EXEC_NS = None


def _bf16(a):
    return np.ascontiguousarray(a.astype(ml_dtypes.bfloat16))


def _f32(a):
    return np.ascontiguousarray(np.asarray(a, dtype=np.float32))


def build_kernel(nonzero_bv, nonzero_bo, profile_mode=False):
    """Builds the Bass program (shared by all 8 cores)."""
    nc = bacc.Bacc("TRN2", target_bir_lowering=False, debug=False,
                   num_devices=1 if profile_mode else NCORES,
                   enable_asserts=False)

    # ---------------- I/O declarations ----------------
    x_in = nc.dram_tensor("x_in", [NT, P, D], F32, kind="ExternalInput")
    keep_in = nc.dram_tensor("keep_in", [NT, P, 1], F32, kind="ExternalInput")
    cosf_in = nc.dram_tensor("cosf_in", [KD, P, T], BF16, kind="ExternalInput")
    sinf_in = nc.dram_tensor("sinf_in", [KD, P, T], BF16, kind="ExternalInput")
    cost_in = nc.dram_tensor("cost_in", [NT, P, D], BF16, kind="ExternalInput")
    sint_in = nc.dram_tensor("sint_in", [NT, P, D], BF16, kind="ExternalInput")
    # stationary weight chunk layouts [m, k, p, f] so chunk (m,k) is [128,128]
    wq_in = nc.dram_tensor("wq_in", [KD, KD, P, P], BF16, kind="ExternalInput")
    # moving weight layouts [p, k, n]
    wkm_in = nc.dram_tensor("wkm_in", [P, KD, D], BF16, kind="ExternalInput")
    wvm_in = nc.dram_tensor("wvm_in", [P, KD, D], BF16, kind="ExternalInput")
    wom_in = nc.dram_tensor("wom_in", [P, KD, D], BF16, kind="ExternalInput")
    # FFN stationary layouts
    w1_in = nc.dram_tensor("w1_in", [P, KF, KD, P], BF16, kind="ExternalInput")
    w2_in = nc.dram_tensor("w2_in", [P, KD, KF, P], BF16, kind="ExternalInput")
    # per-partition biases for feature-major evictions
    bq_in = nc.dram_tensor("bq_in", [P, KD], F32, kind="ExternalInput")
    bk_in = nc.dram_tensor("bk_in", [P, KD], F32, kind="ExternalInput")
    b1_in = nc.dram_tensor("b1_in", [P, KF], F32, kind="ExternalInput")
    b2_in = nc.dram_tensor("b2_in", [P, KD], F32, kind="ExternalInput")
    # free-axis bias vectors (used only if nonzero)
    bv_in = nc.dram_tensor("bv_in", [1, D], F32, kind="ExternalInput")
    bo_in = nc.dram_tensor("bo_in", [1, D], F32, kind="ExternalInput")

    x2_out = nc.dram_tensor("x2_out", [NT, P, D], F32, kind="ExternalOutput")
    y_out = nc.dram_tensor("y_out", [KD, P, T], F32, kind="ExternalOutput")

    rg = None if profile_mode else [[0, 1], [2, 3], [4, 5], [6, 7]]

    with tile.TileContext(nc) as tc:
        with tc.tile_pool(name="dram", bufs=1, space="DRAM") as dram:
            q2_dram = dram.tile([H, P, T], BF16)
            h_dram = dram.tile([KF, P, T], BF16)
            cc_in = dram.tile([P, H * 65], F32)
            cc_out = dram.tile([P, H * 65], F32)

            with tc.tile_pool(name="const", bufs=1) as const:
                ident = const.tile([P, P], BF16)
                make_identity(nc, ident)
                ones12 = const.tile([P, H], BF16)
                nc.vector.memset(ones12, 1.0)
                eps_sb = const.tile([P, 1], F32)
                nc.vector.memset(eps_sb, LN_EPS)
                bq_sb = const.tile([P, KD], F32)
                nc.sync.dma_start(out=bq_sb, in_=bq_in[:])
                bk_sb = const.tile([P, KD], F32)
                nc.sync.dma_start(out=bk_sb, in_=bk_in[:])
                b1_sb = const.tile([P, KF], F32)
                nc.sync.dma_start(out=b1_sb, in_=b1_in[:])
                b2_sb = const.tile([P, KD], F32)
                nc.sync.dma_start(out=b2_sb, in_=b2_in[:])
                kv_bf = const.tile([P, H, 65], BF16)
                if nonzero_bv:
                    bv_bc = const.tile([P, D], F32)
                    nc.sync.dma_start(
                        out=bv_bc, in_=bv_in[:].to_broadcast((P, D)))
                if nonzero_bo:
                    bo_bc = const.tile([P, D], F32)
                    nc.sync.dma_start(
                        out=bo_bc, in_=bo_in[:].to_broadcast((P, D)))

                _build_body(
                    nc, tc, rg,
                    x_in, keep_in, cosf_in, sinf_in, cost_in, sint_in,
                    wq_in, wkm_in, wvm_in, wom_in,
                    bq_sb, bk_sb, b1_sb, b2_sb,
                    bv_bc if nonzero_bv else None,
                    bo_bc if nonzero_bo else None,
                    ident, ones12, eps_sb, kv_bf,
                    q2_dram, h_dram, cc_in, cc_out,
                    w1_in, w2_in,
                    x2_out, y_out,
                )

    nc.compile()
    return nc


def _layernorm(nc, stat, x_t, xn_t, width, eps_sb):
    """token-major LN: x_t [P, width] f32 -> xn_t [P, width] bf16 (normalized,
    no affine -- affine folded into downstream weights)."""
    nsum = stat.tile([P, 1], F32, tag="nsum")
    nc.vector.tensor_reduce(nsum, x_t, axis=mybir.AxisListType.X,
                            op=ALU.add, negate=True)
    sq = stat.tile([P, width], F32, tag="sq")
    sumsq = stat.tile([P, 1], F32, tag="sumsq")
    nc.scalar.activation(sq, x_t, AF.Square, accum_out=sumsq)
    nmean = stat.tile([P, 1], F32, tag="nmean")
    nc.vector.tensor_scalar_mul(nmean, nsum, 1.0 / width)
    ex2 = stat.tile([P, 1], F32, tag="ex2")
    nc.vector.tensor_scalar_mul(ex2, sumsq, 1.0 / width)
    var = stat.tile([P, 1], F32, tag="var")
    # var = ex2 - mean^2 = (nmean * nmean) ... subtract reversed:
    # nmean*nmean = mean^2 ; var = ex2 - mean^2
    m2 = stat.tile([P, 1], F32, tag="m2")
    nc.vector.tensor_mul(m2, nmean, nmean)
    nc.vector.tensor_sub(var, ex2, m2)
    std = stat.tile([P, 1], F32, tag="std")
    nc.scalar.activation(std, var, AF.Sqrt, bias=eps_sb[:])
    rstd = stat.tile([P, 1], F32, tag="rstd")
    nc.vector.reciprocal(rstd, std)
    nmr = stat.tile([P, 1], F32, tag="nmr")
    nc.vector.tensor_mul(nmr, nmean, rstd)
    nc.vector.tensor_scalar(xn_t, x_t, rstd, nmr, op0=ALU.mult, op1=ALU.add)


def _build_body(nc, tc, rg,
                x_in, keep_in, cosf_in, sinf_in, cost_in, sint_in,
                wq_in, wkm_in, wvm_in, wom_in,
                bq_sb, bk_sb, b1_sb, b2_sb, bv_bc, bo_bc,
                ident, ones12, eps_sb, kv_bf,
                q2_dram, h_dram, cc_in, cc_out,
                w1_in, w2_in,
                x2_out, y_out):
    import contextlib

    # ============ Stage A: P1 (LN1) + P2a (q2) + P2b (k2/v/kv) ============
    # P2a units are interleaved into the P2b tile loop so the tensor engine
    # stays busy while P2b's eviction chains run on DVE/ACT; a tail of P2a
    # units is emitted after the collective to cover its latency.
    with contextlib.ExitStack() as stA:
        xn_pool = stA.enter_context(tc.tile_pool(name="xnfm", bufs=1))
        xn_fm = xn_pool.tile([P, KD, T], BF16)

        wmov = stA.enter_context(tc.tile_pool(name="wmov", bufs=1))
        wk_mv = wmov.tile([P, KD, D], BF16)
        nc.sync.dma_start(out=wk_mv, in_=wkm_in[:])
        wv_mv = wmov.tile([P, KD, D], BF16)
        nc.sync.dma_start(out=wv_mv, in_=wvm_in[:])

        # psum pools (8 banks total): tp1 1 + p2a 1 + pvk 3 + pkv 3
        tp1 = stA.enter_context(
            tc.tile_pool(name="tp1", bufs=1, space="PSUM"))
        p2a = stA.enter_context(
            tc.tile_pool(name="p2a", bufs=1, space="PSUM"))
        pvk = stA.enter_context(
            tc.tile_pool(name="pvk", bufs=3, space="PSUM"))
        pkv = stA.enter_context(
            tc.tile_pool(name="pkv", bufs=1, space="PSUM"))

        ldw = stA.enter_context(tc.tile_pool(name="ldw", bufs=6))
        io_a = stA.enter_context(tc.tile_pool(name="io_a", bufs=3))
        stat = stA.enter_context(tc.tile_pool(name="statA", bufs=3))
        work = stA.enter_context(tc.tile_pool(name="workA", bufs=3))

        # ---- P1: LayerNorm 1, per token tile -> xn_fm ----
        for t in range(NT):
            x_t = io_a.tile([P, D], F32, tag="x")
            nc.sync.dma_start(out=x_t, in_=x_in[t])
            xn_t = work.tile([P, D], BF16, tag="xn_tok")
            _layernorm(nc, stat, x_t, xn_t, D, eps_sb)
            for k in range(KD):
                tp = tp1.tile([P, P], BF16, tag="tp")
                nc.tensor.transpose(tp, xn_t[:, ts(k, P)], ident)
                nc.any.tensor_copy(xn_fm[:, k, ts(t, P)], tp)

        def emit_p2a(n, m):
            nsl = ts(n, 512)
            ps = p2a.tile([P, 512], F32, tag="q")
            for k in range(KD):
                wq_t = ldw.tile([P, P], BF16, tag="wq")
                nc.sync.dma_start(out=wq_t, in_=wq_in[m, k])
                nc.tensor.matmul(ps, wq_t, xn_fm[:, k, nsl],
                                 start=(k == 0), stop=(k == KD - 1))
            q_t = work.tile([P, 512], BF16, tag="q_fm")
            nc.scalar.activation(q_t, ps, AF.Relu, bias=bq_sb[:, ds(m, 1)])
            cf = io_a.tile([P, 512], BF16, tag="cosf")
            nc.sync.dma_start(out=cf, in_=cosf_in[m, :, nsl])
            sf = io_a.tile([P, 512], BF16, tag="sinf")
            nc.sync.dma_start(out=sf, in_=sinf_in[m, :, nsl])
            qc = work.tile([P, 512], BF16, tag="q2c")
            nc.vector.tensor_mul(qc, q_t, cf)
            qs = work.tile([P, 512], BF16, tag="q2s")
            nc.vector.tensor_mul(qs, q_t, sf)
            # pack: head h features = [cos(64) | sin(64)]
            h0, h1 = 2 * m, 2 * m + 1
            nc.sync.dma_start(out=q2_dram[h0, ds(0, 64), nsl],
                              in_=qc[ds(0, 64), :])
            nc.sync.dma_start(out=q2_dram[h1, ds(0, 64), nsl],
                              in_=qc[ds(64, 64), :])
            nc.sync.dma_start(out=q2_dram[h0, ds(64, 64), nsl],
                              in_=qs[ds(0, 64), :])
            nc.sync.dma_start(out=q2_dram[h1, ds(64, 64), nsl],
                              in_=qs[ds(64, 64), :])

        kv_ps = pkv.tile([P, H, P], F32)  # head h uses cols [h, 0:65]

        def emit_p2b(t):
            tsl = ts(t, P)
            keep_t = io_a.tile([P, 1], F32, tag="keep")
            nc.sync.dma_start(out=keep_t, in_=keep_in[t])
            ct = io_a.tile([P, D], BF16, tag="cost")
            nc.sync.dma_start(out=ct, in_=cost_in[t])
            st = io_a.tile([P, D], BF16, tag="sint")
            nc.sync.dma_start(out=st, in_=sint_in[t])

            v_aug = work.tile([P, H, 65], BF16, tag="vaug")
            k_tok = work.tile([P, D], BF16, tag="ktok")
            # v and k in two psum chunks each (cols 0:512, 512:768)
            for c0, cw in ((0, 512), (512, 256)):
                psv = pvk.tile([P, 512], F32, tag="vk")
                for k in range(KD):
                    nc.tensor.matmul(psv[:, ds(0, cw)],
                                     xn_fm[:, k, tsl],
                                     wv_mv[:, k, ds(c0, cw)],
                                     start=(k == 0), stop=(k == KD - 1))
                # v_aug[:, h, 0:64] = (psv + bv) * keep  (ACT: copy w/ scale)
                nh = cw // 64
                vsrc = psv[:, ds(0, cw)].rearrange("p (h f) -> p h f", f=64)
                vdst = v_aug[:, ds(c0 // 64, nh), ds(0, 64)]
                if bv_bc is not None:
                    tmp = work.tile([P, 512], F32, tag="vtmp")
                    nc.vector.tensor_add(tmp[:, ds(0, cw)], psv[:, ds(0, cw)],
                                         bv_bc[:, ds(c0, cw)])
                    nc.scalar.mul(vdst, tmp[:, ds(0, cw)].rearrange(
                        "p (h f) -> p h f", f=64), keep_t[:])
                else:
                    nc.scalar.mul(vdst, vsrc, keep_t[:])

                psk = pvk.tile([P, 512], F32, tag="vk")
                for k in range(KD):
                    nc.tensor.matmul(psk[:, ds(0, cw)],
                                     xn_fm[:, k, tsl],
                                     wk_mv[:, k, ds(c0, cw)],
                                     start=(k == 0), stop=(k == KD - 1))
                nc.scalar.activation(k_tok[:, ds(c0, cw)], psk[:, ds(0, cw)],
                                     AF.Relu)
            # keep column
            nc.scalar.mul(v_aug[:, :, ds(64, 1)].opt(), ones12[:], keep_t[:])
            # k2 = [k*cos, k*sin] packed per head
            k2_t = work.tile([P, H, P], BF16, tag="k2")
            nc.vector.tensor_mul(
                k2_t[:, :, ds(0, 64)],
                k_tok[:].rearrange("p (h f) -> p h f", f=64),
                ct[:].rearrange("p (h f) -> p h f", f=64))
            nc.vector.tensor_mul(
                k2_t[:, :, ds(64, 64)],
                k_tok[:].rearrange("p (h f) -> p h f", f=64),
                st[:].rearrange("p (h f) -> p h f", f=64))
            for h in range(H):
                nc.tensor.matmul(kv_ps[:, h, ds(0, 65)],
                                 k2_t[:, h, :], v_aug[:, h, :],
                                 start=(t == 0), stop=(t == NT - 1),
                                 skip_group_check=True)

        # interleaved emission: P2b tiles with P2a units sprinkled in;
        # hold back the last P2A_TAIL units to cover the collective.
        p2a_units = [(n, m) for n in range(NQ) for m in range(KD)]
        P2A_TAIL = 8
        ui = 0
        for t in range(NT):
            emit_p2b(t)
            target = ((t + 1) * (len(p2a_units) - P2A_TAIL)) // NT
            while ui < target:
                emit_p2a(*p2a_units[ui])
                ui += 1

        # ---- P3: collective ----
        kv_f = work.tile([P, H * 65], F32, tag="kvf")
        nc.any.tensor_copy(kv_f[:].rearrange("p (h f) -> p h f", f=65),
                           kv_ps[:, :, ds(0, 65)])
        nc.sync.dma_start(out=cc_in[:], in_=kv_f[:])
        if rg is None:  # profile mode: stand-in DMA instead of AllReduce
            nc.sync.dma_start(out=cc_out[:], in_=cc_in[:])
        else:
            nc.gpsimd.collective_compute(
                "AllReduce", ALU.add, replica_groups=rg,
                ins=[cc_in[:].opt()], outs=[cc_out[:].opt()])

        # tail P2a units: independent of the collective, keep PE busy
        while ui < len(p2a_units):
            emit_p2a(*p2a_units[ui])
            ui += 1

        kv_f2 = work.tile([P, H * 65], F32, tag="kvf2")
        nc.sync.dma_start(out=kv_f2, in_=cc_out[:])
        nc.any.tensor_copy(kv_bf[:],
                           kv_f2[:].rearrange("p (h f) -> p h f", f=65))

    # ====== Stage B: P4 attn + Wo + residual + LN2, P5 FFN1 interleaved ====
    with contextlib.ExitStack() as stBC:
        xn2_pool = stBC.enter_context(tc.tile_pool(name="xn2fm", bufs=1))
        xn2_fm = xn2_pool.tile([P, KD, T], BF16)
        w12 = stBC.enter_context(tc.tile_pool(name="w12", bufs=1))
        w1_sb = w12.tile([P, KF, KD, P], BF16)
        nc.sync.dma_start(out=w1_sb, in_=w1_in[:])
        w2_sb = w12.tile([P, KD, KF, P], BF16)
        nc.sync.dma_start(out=w2_sb, in_=w2_in[:])

        workC = stBC.enter_context(tc.tile_pool(name="workC", bufs=3))

        stB = stBC.enter_context(contextlib.ExitStack())
        womv_p = stB.enter_context(tc.tile_pool(name="womv", bufs=1))
        wo_mv = womv_p.tile([P, KD, D], BF16)
        nc.sync.dma_start(out=wo_mv, in_=wom_in[:])

        # psum: pat 3 + po 2 + tp4 1 + p5 2 = 8 banks
        pat = stB.enter_context(tc.tile_pool(name="pat", bufs=1, space="PSUM"))
        po = stB.enter_context(tc.tile_pool(name="po", bufs=1, space="PSUM"))
        tp4 = stB.enter_context(tc.tile_pool(name="tp4", bufs=1, space="PSUM"))
        p5 = stB.enter_context(tc.tile_pool(name="p5", bufs=2, space="PSUM"))

        q2io = stB.enter_context(tc.tile_pool(name="q2io", bufs=16))
        io_b = stB.enter_context(tc.tile_pool(name="io_b", bufs=3))
        statB = stB.enter_context(tc.tile_pool(name="statB", bufs=3))
        workB = stB.enter_context(tc.tile_pool(name="workB", bufs=3))

        def emit_p4(t):
            tsl = ts(t, P)
            at_ps = pat.tile([P, H, P], F32, tag="at")
            for h in range(H):
                q2_t = q2io.tile([P, P], BF16, tag="q2")
                nc.sync.dma_start(out=q2_t, in_=q2_dram[h, :, tsl])
                nc.tensor.matmul(at_ps[:, h, ds(0, 65)], q2_t,
                                 kv_bf[:, h, :], start=True, stop=True)
            # z = 1 / (s + eps)
            s_t = statB.tile([P, H], F32, tag="s")
            nc.vector.tensor_scalar_add(s_t, at_ps[:, :, ds(64, 1)].opt(),
                                        DENOM_EPS)
            z_t = statB.tile([P, H], F32, tag="z")
            nc.vector.reciprocal(z_t, s_t)
            attn_tok = workB.tile([P, D], BF16, tag="attn_tok")
            for h in range(H):
                nc.vector.tensor_scalar(
                    attn_tok[:, ds(h * 64, 64)], at_ps[:, h, ds(0, 64)],
                    z_t[:, ds(h, 1)], None, op0=ALU.mult)
            # transpose attn -> feature-major
            attn_fm = workB.tile([P, KD, P], BF16, tag="attn_fm")
            for k in range(KD):
                tp = tp4.tile([P, P], BF16, tag="tp")
                nc.tensor.transpose(tp, attn_tok[:, ts(k, P)], ident)
                nc.any.tensor_copy(attn_fm[:, k, :], tp)
            # Wo: token-major out
            o_ps = po.tile([P, 2, 512], F32, tag="o")
            for k in range(KD):
                nc.tensor.matmul(o_ps[:, 0, :], attn_fm[:, k, :],
                                 wo_mv[:, k, ds(0, 512)],
                                 start=(k == 0), stop=(k == KD - 1))
                nc.tensor.matmul(o_ps[:, 1, ds(0, 256)], attn_fm[:, k, :],
                                 wo_mv[:, k, ds(512, 256)],
                                 start=(k == 0), stop=(k == KD - 1))
            x_t = io_b.tile([P, D], F32, tag="x")
            nc.sync.dma_start(out=x_t, in_=x_in[t])
            x2_t = workB.tile([P, D], F32, tag="x2")
            if bo_bc is not None:
                nc.any.tensor_add(x_t, x_t, bo_bc)
            nc.any.tensor_add(x2_t[:, ds(0, 512)], o_ps[:, 0, :],
                              x_t[:, ds(0, 512)])
            nc.any.tensor_add(x2_t[:, ds(512, 256)], o_ps[:, 1, ds(0, 256)],
                              x_t[:, ds(512, 256)])
            nc.sync.dma_start(out=x2_out[t], in_=x2_t)
            # LN2 -> xn2 feature-major
            xn2_t = workB.tile([P, D], BF16, tag="xn2_tok")
            _layernorm(nc, statB, x2_t, xn2_t, D, eps_sb)
            for k in range(KD):
                tp = tp4.tile([P, P], BF16, tag="tp")
                nc.tensor.transpose(tp, xn2_t[:, ts(k, P)], ident)
                nc.any.tensor_copy(xn2_fm[:, k, ts(t, P)], tp)

        def emit_p5(n):
            # h = gelu(xn2 @ W1 + b1), feature-major out, staged to DRAM
            nsl = ts(n, 512)
            for m in range(KF):
                ps = p5.tile([P, 512], F32, tag="h")
                for k in range(KD):
                    nc.tensor.matmul(ps, w1_sb[:, m, k, :],
                                     xn2_fm[:, k, nsl],
                                     start=(k == 0), stop=(k == KD - 1))
                h_t = workC.tile([P, 512], BF16, tag="h")
                nc.scalar.activation(h_t, ps, AF.Gelu,
                                     bias=b1_sb[:, ds(m, 1)])
                nc.sync.dma_start(out=h_dram[m, :, nsl], in_=h_t)

        for t in range(NT):
            emit_p4(t)
            if t % 4 == 3:
                emit_p5(t // 4)

        # close Stage B pools (frees PSUM banks) before P6
        stB.close()

        # ============ Stage C: P6 FFN2 ============
        with contextlib.ExitStack() as stC:
            p6 = stC.enter_context(
                tc.tile_pool(name="p6", bufs=3, space="PSUM"))
            hio = stC.enter_context(tc.tile_pool(name="hio", bufs=28))

            # P6: y = h @ W2 (+ b2), feature-major out -> y_out
            for n in range(NQ):
                nsl = ts(n, 512)
                h_sb = []
                for k in range(KF):
                    h_k = hio.tile([P, 512], BF16, tag="hin")
                    nc.sync.dma_start(out=h_k, in_=h_dram[k, :, nsl])
                    h_sb.append(h_k)
                for m in range(KD):
                    ps = p6.tile([P, 512], F32, tag="y")
                    for k in range(KF):
                        nc.tensor.matmul(ps, w2_sb[:, m, k, :], h_sb[k][:],
                                         start=(k == 0), stop=(k == KF - 1))
                    y_t = workC.tile([P, 512], F32, tag="y")
                    nc.scalar.activation(y_t, ps, AF.Identity,
                                         bias=b2_sb[:, ds(m, 1)])
                    nc.sync.dma_start(out=y_out[m, :, nsl], in_=y_t)


def _prep_shared(inputs):
    """Host-side weight prep: fold LN affine, build device layouts."""
    g1 = _f32(inputs["g1"]); be1 = _f32(inputs["be1"])
    g2 = _f32(inputs["g2"]); be2 = _f32(inputs["be2"])
    Wq = _f32(inputs["Wq"]); Wk = _f32(inputs["Wk"]); Wv = _f32(inputs["Wv"])
    Wo = _f32(inputs["Wo"]); W1 = _f32(inputs["W1"]); W2 = _f32(inputs["W2"])
    bq = _f32(inputs["bq"]); bk = _f32(inputs["bk"]); bv = _f32(inputs["bv"])
    bo = _f32(inputs["bo"]); b1 = _f32(inputs["b1"]); b2 = _f32(inputs["b2"])

    Wq_f = g1[:, None] * Wq; bq_f = be1 @ Wq + bq
    Wk_f = g1[:, None] * Wk; bk_f = be1 @ Wk + bk
    Wv_f = g1[:, None] * Wv; bv_f = be1 @ Wv + bv
    W1_f = g2[:, None] * W1; b1_f = be2 @ W1 + b1

    d = {}
    # stationary [m, k, p, f]
    d["wq_in"] = _bf16(Wq_f.reshape(KD, P, KD, P).transpose(2, 0, 1, 3))
    # moving [p, k, n]
    d["wkm_in"] = _bf16(Wk_f.reshape(KD, P, D).transpose(1, 0, 2))
    d["wvm_in"] = _bf16(Wv_f.reshape(KD, P, D).transpose(1, 0, 2))
    d["wom_in"] = _bf16(Wo.reshape(KD, P, D).transpose(1, 0, 2))
    # w1 stationary [p, m, k, f]: element = W1_f[k*128+p, m*128+f]
    d["w1_in"] = _bf16(W1_f.reshape(KD, P, KF, P).transpose(1, 2, 0, 3))
    # w2 stationary [p, m, k, f]: element = W2[k*128+p, m*128+f]
    d["w2_in"] = _bf16(W2.reshape(KF, P, KD, P).transpose(1, 2, 0, 3))
    d["bq_in"] = _f32(bq_f.reshape(KD, P).T)
    d["bk_in"] = _f32(bk_f.reshape(KD, P).T)
    d["b1_in"] = _f32(b1_f.reshape(KF, P).T)
    d["b2_in"] = _f32(b2.reshape(KD, P).T)
    d["bv_in"] = _f32(bv_f.reshape(1, D))
    d["bo_in"] = _f32(bo.reshape(1, D))
    nonzero_bv = bool(np.abs(bv_f).max() > 0)
    nonzero_bo = bool(np.abs(bo).max() > 0)
    return d, nonzero_bv, nonzero_bo


def kernel(**inputs):
    global LAST_EXEC_NS
    x = _f32(inputs["x"])                      # [B, S, D]
    mask = np.asarray(inputs["mask"])          # [B, S, 1] bool
    cos = _f32(inputs["cos"]).reshape(B, S, D)
    sin = _f32(inputs["sin"]).reshape(B, S, D)
    keep = (~mask.astype(bool)).astype(np.float32)  # [B, S, 1]

    shared, nonzero_bv, nonzero_bo = _prep_shared(inputs)

    key = ("kern", nonzero_bv, nonzero_bo)
    if key not in _CACHE:
        _CACHE[key] = build_kernel(nonzero_bv, nonzero_bo)
    nc = _CACHE[key]

    in_maps = []
    for c in range(NCORES):
        b, half = divmod(c, 2)
        s0 = half * T
        sl = slice(s0, s0 + T)
        m = dict(shared)
        m["x_in"] = _f32(x[b, sl].reshape(NT, P, D))
        m["keep_in"] = _f32(keep[b, sl].reshape(NT, P, 1))
        cbs = cos[b, sl]; sbs = sin[b, sl]
        m["cosf_in"] = _bf16(cbs.T.reshape(KD, P, T))
        m["sinf_in"] = _bf16(sbs.T.reshape(KD, P, T))
        m["cost_in"] = _bf16(cbs.reshape(NT, P, D))
        m["sint_in"] = _bf16(sbs.reshape(NT, P, D))
        in_maps.append(m)

    if bool(int(os.environ.get("KERNEL_TRACE", "0"))):
        results = _run_traced(nc, in_maps)
    else:
        results = _run_pjrt_timed(
            nc, in_maps,
            n_timed=int(os.environ.get("KERNEL_TIMED_ITERS", "0")))

    out = np.empty((B, S, D), np.float32)
    for c in range(NCORES):
        b, half = divmod(c, 2)
        s0 = half * T
        r = results[c]
        x2 = r["x2_out"].reshape(T, D)
        y = r["y_out"].reshape(D, T).T
        out[b, s0:s0 + T] = x2 + y
    return out


def _enable_ntff_hook():
    """Inject the missing antenv.axon_hooks shim so run_bass_kernel_spmd's
    trace=True path can reach the libaxon NTFF profiling C ABI."""
    import sys
    import types
    if "antenv.axon_hooks" in sys.modules:
        return
    mod = types.ModuleType("antenv.axon_hooks")
    state = {"hook": None}
    mod.set_axon_ntff_profile_hook = lambda h: state.__setitem__("hook", h)
    mod.get_axon_ntff_profile_hook = lambda: state["hook"]
    sys.modules["antenv.axon_hooks"] = mod
    from trn_agent_boot.trn_boot import _ntff_profile_via_ctypes
    mod.set_axon_ntff_profile_hook(
        _ntff_profile_via_ctypes("/opt/axon/libaxon_pjrt.so"))
    # skip the artifact upload (no bucket credentials in this container)
    bass_utils.upload_artifacts = lambda tmpdir: str(tmpdir)


def _run_traced(nc, in_maps):
    global LAST_EXEC_NS
    _enable_ntff_hook()
    tmpdir = os.environ.get("KERNEL_TRACE_DIR")
    if tmpdir:
        os.makedirs(tmpdir, exist_ok=True)
    res = bass_utils.run_bass_kernel_spmd(
        nc, in_maps, core_ids=list(range(NCORES)), trace=True,
        tmpdir=tmpdir)
    LAST_EXEC_NS = res.exec_time_ns
    return res.results


def _run_pjrt_timed(nc, in_maps, n_timed=0):
    """Replicates bass2jax.run_bass_via_pjrt's multi-core path, with inputs
    pre-transferred via device_put so optional repeat timing excludes H2D."""
    global LAST_EXEC_NS
    import time
    import jax
    from jax.sharding import Mesh, PartitionSpec, NamedSharding
    from jax.experimental.shard_map import shard_map
    from concourse import bass2jax, mybir as mb

    bass2jax.install_neuronx_cc_hook()
    partition_name = (nc.partition_id_tensor.name
                      if nc.partition_id_tensor else None)

    in_names, out_names, out_avals, zero_outs = [], [], [], []
    for alloc in nc.m.functions[0].allocations:
        if not isinstance(alloc, mb.MemoryLocationSet):
            continue
        name = alloc.memorylocations[0].name
        if alloc.kind == "ExternalInput":
            if name != partition_name:
                in_names.append(name)
        elif alloc.kind == "ExternalOutput":
            out_names.append(name)
            shape = tuple(alloc.tensor_shape)
            dtype = mb.dt.np(alloc.dtype)
            out_avals.append(jax.core.ShapedArray(shape, dtype))
            zero_outs.append(np.zeros(shape, dtype))
    n_params = len(in_names)
    n_outs = len(out_avals)
    all_in_names = list(in_names) + out_names
    if partition_name is not None:
        all_in_names.append(partition_name)

    def _body(*args):
        operands = list(args)
        if partition_name is not None:
            operands.append(bass2jax.partition_id_tensor())
        outs = bass2jax._bass_exec_p.bind(
            *operands,
            out_avals=tuple(out_avals),
            in_names=tuple(all_in_names),
            out_names=tuple(out_names),
            lowering_input_output_aliases=(),
            sim_require_finite=True,
            sim_require_nnan=True,
            nc=nc,
        )
        return tuple(outs)

    devices = jax.devices()[:NCORES]
    mesh = Mesh(np.asarray(devices), ("core",))
    in_specs = (PartitionSpec("core"),) * (n_params + n_outs)
    out_specs = (PartitionSpec("core"),) * n_outs
    sharded = jax.jit(
        shard_map(_body, mesh=mesh, in_specs=in_specs, out_specs=out_specs,
                  check_rep=False),
        donate_argnums=tuple(range(n_params, n_params + n_outs)),
        keep_unused=True,
    )
    shard = NamedSharding(mesh, PartitionSpec("core"))
    concat_in = [
        jax.device_put(
            np.concatenate([np.asarray(in_maps[c][n]) for c in range(NCORES)],
                           axis=0), shard)
        for n in in_names
    ]

    def _zeros():
        return [jax.device_put(
            np.zeros((NCORES * z.shape[0], *z.shape[1:]), z.dtype), shard)
            for z in zero_outs]

    out_arrs = sharded(*concat_in, *_zeros())
    jax.block_until_ready(out_arrs)

    if n_timed > 0:
        best = float("inf")
        for _ in range(n_timed):
            zs = _zeros()
            jax.block_until_ready(zs)
            t0 = time.perf_counter()
            o = sharded(*concat_in, *zs)
            jax.block_until_ready(o)
            best = min(best, time.perf_counter() - t0)
            out_arrs = o
        LAST_EXEC_NS = int(best * 1e9)

    return [
        {name: np.asarray(out_arrs[i]).reshape(NCORES, *out_avals[i].shape)[c]
         for i, name in enumerate(out_names)}
        for c in range(NCORES)
    ]
